# revision 16
# baseline (speedup 1.0000x reference)
"""Trainium2 Bass kernel for nn_AnchorKNN_OnlyL (retrieval_knn).

Per query b (32768 total): among its own 2048 2-D anchors, find the 8 nearest
(L2), run each through a 2->128->128 erf-GELU MLP, and combine with
softmax(d2_top / tau) weights.

Sharding: pure data parallel over queries -- 4096 queries per NeuronCore,
params replicated, no collectives.

v2 design (supertiles of 4 x 128 queries):
  - host passes anchors twice: planar [2, bp, M] for contiguous distance
    reads, interleaved [bp, M, 2] for the 256B-chunk gather
  - selection path in bf16: ACT squares -> DVE STT add -> MAX8/FIND_INDEX8
  - all K-wide small ops batched 4 tiles wide ([P, 32])
  - wrapped gather index list built with a DRAM bounce (2 small DMAs)
    instead of the PE transpose fold
  - one dma_gather of 4096 indices per supertile
  - softmax weights broadcast to [EMB, K*P] via ones[1,EMB] matmul into PSUM
  - MLP layer1 as 8 contraction-2 bf16 matmuls from a single batched
    topAT transpose; layer2 bf16 128x128; k-fold via strided tensor_reduce
"""

import sys

sys.path.insert(0, "/opt/trn_rl_repo")

import numpy as np

B, M, EMB, K = 32768, 2048, 128, 8
TAU = 0.3
NCORES = 8
BP = B // NCORES          # queries per core
P = 128                   # partitions / queries per tile
NT_FULL = BP // P         # 32 tiles per core
ST = 4                    # tiles per supertile
NS_FULL = NT_FULL // ST   # 8 supertiles per core
CHF = 64                  # floats per gathered chunk (256 B = 32 anchors)
NCHUNK = (M * 2) // CHF   # 64 chunks per query row


def host_prep(W1, b1, W2, b2):
    """Derived replicated parameters computed host-side."""
    w1T = np.ascontiguousarray(W1.T)                      # [2, EMB]
    w1big = np.zeros((2 * K, K * EMB), dtype=np.float32)  # 8 block stationaries
    for k in range(K):
        w1big[2 * k : 2 * k + 2, k * EMB : (k + 1) * EMB] = w1T
    w2t = np.ascontiguousarray(W2.T)                      # [EMB, EMB]
    return {
        "w1t": w1big,
        "w2t": w2t,
        "b1c": np.ascontiguousarray(b1.reshape(EMB, 1)),
        "b2c": np.ascontiguousarray(b2.reshape(EMB, 1)),
        "repident": _repident(),
    }


def _repident():
    r = np.zeros((16, P), dtype=np.float32)   # S[r, p] = (p % 16 == r)
    for p in range(P):
        r[p % 16, p] = 1.0
    return r


def shard_queries(Gl_cur, n_tiles):
    """[bp, 2] -> negated, tile-major layout [P, n_tiles, 2]."""
    g = Gl_cur.reshape(n_tiles, P, 2).transpose(1, 0, 2)
    return np.ascontiguousarray(-g)


def build_graph(n_super=NS_FULL):
    import concourse.bass as bass
    import concourse.mybir as mybir
    from concourse.bass import ds, ts
    from concourse.masks import make_identity
    from concourse.tile import TileContext

    f32 = mybir.dt.float32
    bf16 = mybir.dt.bfloat16
    i16 = mybir.dt.int16
    u16 = mybir.dt.uint16
    Alu = mybir.AluOpType
    Act = mybir.ActivationFunctionType

    from concourse import bacc

    n_tiles = n_super * ST
    bp = n_tiles * P
    KS = K * ST               # 32: batched top-k width per supertile
    nc = bacc.Bacc(None, dynamic_dma_scratch_size=16384, num_swdge_queues=4)

    ngl_ext = nc.declare_dram_parameter("ngl", [P, n_tiles, 2], f32, isOutput=False)
    ancp_ext = nc.declare_dram_parameter("ancp", [2, bp, M], f32, isOutput=False)
    anci_ext = nc.declare_dram_parameter("anci", [bp, M, 2], f32, isOutput=False)
    w1t_ext = nc.declare_dram_parameter("w1t", [2 * K, K * EMB], f32, isOutput=False)
    w2t_ext = nc.declare_dram_parameter("w2t", [EMB, EMB], f32, isOutput=False)
    b1_ext = nc.declare_dram_parameter("b1c", [EMB, 1], f32, isOutput=False)
    b2_ext = nc.declare_dram_parameter("b2c", [EMB, 1], f32, isOutput=False)
    repident_ext = nc.declare_dram_parameter("repident", [16, P], f32, isOutput=False)
    out_ext = nc.declare_dram_parameter("out", [bp, EMB], f32, isOutput=True)

    with TileContext(nc) as tc:
        with (
            tc.tile_pool(name="consts", bufs=1) as consts,
            tc.tile_pool(name="xy", bufs=6) as xy_pool,
            tc.tile_pool(name="sq", bufs=6) as sq_pool,
            tc.tile_pool(name="s16", bufs=4) as s_pool,
            tc.tile_pool(name="sel", bufs=4) as sel_pool,
            tc.tile_pool(name="mid", bufs=2) as mid_pool,
            tc.tile_pool(name="mlp", bufs=5) as mlp_pool,
            tc.tile_pool(name="psum_mm", bufs=2, space="PSUM") as ppool,
            tc.tile_pool(name="psum_tp", bufs=1, space="PSUM") as tpool,
            tc.tile_pool(name="psum_f1", bufs=1, space="PSUM") as f1pool,
            tc.tile_pool(name="psum_f2", bufs=1, space="PSUM") as f2pool,
            tc.tile_pool(name="psum_oq", bufs=1, space="PSUM") as oqpool,
            tc.tile_pool(name="dram", bufs=4, space="DRAM") as dram_pool,
        ):
            # ---------------- constants ----------------
            ident = consts.tile([P, P], f32)
            make_identity(nc, ident)
            identb = consts.tile([P, P], bf16)
            nc.scalar.copy(identb, ident)

            w1sb = consts.tile([2 * K, K * EMB], f32)
            nc.sync.dma_start(out=w1sb, in_=w1t_ext[:, :])
            w1T = consts.tile([2 * K, K * EMB], bf16)
            nc.scalar.copy(w1T, w1sb)

            w2sb = consts.tile([EMB, EMB], f32)
            nc.sync.dma_start(out=w2sb, in_=w2t_ext[:, :])
            w2T = consts.tile([EMB, EMB], bf16)
            nc.scalar.copy(w2T, w2sb)

            b1c = consts.tile([EMB, 1], f32)
            nc.sync.dma_start(out=b1c, in_=b1_ext[:, :])
            b2c = consts.tile([EMB, 1], f32)
            nc.sync.dma_start(out=b2c, in_=b2_ext[:, :])

            repident = consts.tile([16, P], f32)
            nc.sync.dma_start(out=repident, in_=repident_ext[:, :])

            ngl = consts.tile([P, n_tiles, 2], f32)
            nc.sync.dma_start(out=ngl, in_=ngl_ext[:, :, :])

            # jhalf[p, j] = j // 2 for j in [0, 64): pair id of float j in chunk
            jhalf_u = consts.tile([P, CHF], u16)
            nc.gpsimd.iota(jhalf_u, pattern=[[1, CHF // 2], [0, 2]], channel_multiplier=0)
            jhalf = consts.tile([P, CHF], bf16)
            nc.scalar.copy(jhalf, jhalf_u)

            # qiota[p, t*K+k] = 64*p + 8192*t (chunk-row base within supertile)
            qiota = consts.tile([P, KS], u16)
            nc.gpsimd.iota(
                qiota, pattern=[[P * NCHUNK, ST], [0, K]], channel_multiplier=NCHUNK
            )

            def stage_a(s):
                """Per supertile: load 4 tiles, distances, top-8 each."""
                vals8 = sel_pool.tile([P, KS], bf16, tag="vals8")
                idx8 = sel_pool.tile([P, KS], u16, tag="idx8")
                for j in range(ST):
                    t = s * ST + j
                    xt = xy_pool.tile([P, M], f32, tag="xy")
                    nc.sync.dma_start(out=xt, in_=ancp_ext[0, ts(t, P)])
                    yt = xy_pool.tile([P, M], f32, tag="xy")
                    nc.sync.dma_start(out=yt, in_=ancp_ext[1, ts(t, P)])

                    tx2 = sq_pool.tile([P, M], bf16, tag="sq")
                    ty2 = sq_pool.tile([P, M], bf16, tag="sq")
                    nc.scalar.activation(tx2, xt, Act.Square, bias=ngl[:, t, 0:1])
                    nc.scalar.activation(ty2, yt, Act.Square, bias=ngl[:, t, 1:2])

                    st_ = s_pool.tile([P, M], bf16, tag="s16")
                    nc.vector.scalar_tensor_tensor(
                        out=st_, in0=tx2, scalar=-1.0, in1=ty2,
                        op0=Alu.mult, op1=Alu.subtract,
                    )
                    nc.vector.max(out=vals8[:, ts(j, K)], in_=st_)
                    nc.vector.max_index(
                        out=idx8[:, ts(j, K)], in_max=vals8[:, ts(j, K)], in_values=st_
                    )
                return dict(vals8=vals8, idx8=idx8)

            def stage_b1(s, st):
                """Batched: index math, fold, gather, softmax, extraction."""
                vals8, idx8 = st["vals8"], st["idx8"]

                # ---- gather index list ----
                chunk = sel_pool.tile([P, KS], u16, tag="chunk")
                nc.vector.tensor_scalar(
                    out=chunk, in0=idx8, scalar1=5, scalar2=None,
                    op0=Alu.logical_shift_right,
                )
                chunkq = sel_pool.tile([P, KS], u16, tag="chunkq")
                nc.vector.tensor_tensor(out=chunkq, in0=chunk, in1=qiota, op=Alu.add)

                # ---- softmax weights (tanh-exp), batched [P, 32] ----
                v = vals8.rearrange("p (t k) -> p t k", t=ST)
                sub = sel_pool.tile([P, ST, K], f32, tag="sub")
                nc.vector.tensor_tensor(
                    out=sub, in0=v,
                    in1=v[:, :, 7:8].broadcast_to([P, ST, K]),
                    op=Alu.subtract,
                )
                th = sel_pool.tile([P, KS], f32, tag="th")
                nc.scalar.activation(
                    th, sub.rearrange("p t k -> p (t k)"), Act.Tanh,
                    scale=-1.0 / (2.0 * TAU),
                )
                den = sel_pool.tile([P, KS], f32, tag="den")
                nc.vector.tensor_scalar(
                    out=den, in0=th, scalar1=-1.0, scalar2=1.0,
                    op0=Alu.mult, op1=Alu.add,
                )
                rden8 = sel_pool.tile([P, KS], f32, tag="rden8")
                nc.vector.reciprocal(rden8, den)

                # cmb = [chunkq as f32 | exp8]: one transpose serves both the
                # gather-index fold and the softmax-weight broadcast
                cmb = sel_pool.tile([P, 2 * KS], f32, tag="cmb")
                nc.scalar.copy(cmb[:, :KS], chunkq)
                nc.vector.scalar_tensor_tensor(
                    out=cmb[:, ds(KS, KS)], in0=th, scalar=1.0, in1=rden8,
                    op0=Alu.add, op1=Alu.mult,
                )
                denom = sel_pool.tile([P, ST, 1], f32, tag="denom")
                nc.vector.tensor_reduce(
                    out=denom,
                    in_=cmb[:, ds(KS, KS)].rearrange("p (t k) -> p t k", t=ST),
                    axis=mybir.AxisListType.X,
                    op=Alu.add,
                )
                rden = sel_pool.tile([P, ST, 1], f32, tag="rden")
                nc.vector.reciprocal(rden, denom)
                st["rden"] = rden

                # ---- fold: [P, 64] -> [64, P] -> wrapped idx list + expT ----
                cqeT_ps = f1pool.tile([2 * KS, P], f32, tag="f1")
                nc.tensor.transpose(cqeT_ps, cmb, ident)
                cqT_sb = sel_pool.tile([KS, P], f32, tag="cqT_sb")
                nc.scalar.copy(cqT_sb, cqeT_ps[ds(0, KS), :])
                expT_sb = sel_pool.tile([KS, P], bf16, tag="expT_sb")
                nc.scalar.copy(expT_sb, cqeT_ps[ds(KS, KS), :])
                wscr = dram_pool.tile([KS, P], bf16, tag="wscr")
                nc.sync.dma_start(out=wscr, in_=expT_sb)
                st["wscr"] = wscr

                m16_ps = f2pool.tile([16, KS * 8], f32, tag="f2")
                for jj in range(8):
                    nc.tensor.transpose(
                        m16_ps[:, ds(KS * jj, KS)],
                        cqT_sb[:, ds(16 * jj, 16)],
                        ident[ds(0, KS), ds(0, KS)],
                    )
                m16_sb = sel_pool.tile([16, KS * 8], f32, tag="m16_sb")
                nc.scalar.copy(m16_sb, m16_ps)
                wps = ppool.tile([EMB, K * P], f32, tag="pmm")
                nc.tensor.matmul(
                    wps[:, : KS * 8],
                    repident,
                    m16_sb.rearrange("r (j tk) -> r tk j", j=8),
                )
                wrapped = sel_pool.tile([P, KS * 8], i16, tag="wrapped")
                nc.scalar.copy(wrapped, wps[:, : KS * 8])

                # ---- gather 256B chunks, one 1024-idx call per tile ----
                chunks = mid_pool.tile([P, KS, CHF], f32, tag="chunks")
                for j in range(ST):
                    nc.gpsimd.dma_gather(
                        out_ap=chunks[:, ds(K * j, K), :],
                        in_ap=anci_ext[ts(s, ST * P)].rearrange(
                            "p (g r) c -> (p g) (r c)", r=CHF // 2
                        ),
                        idxs_ap=wrapped[:, ds(K * 8 * j, K * 8)],
                        num_idxs=P * K,
                        num_idxs_reg=P * K,
                        elem_size=CHF,
                        queue_num=j,
                    )

                # ---- chunk-local selection mask ----
                loc_u = sel_pool.tile([P, KS], u16, tag="loc_u")
                nc.vector.tensor_scalar(
                    out=loc_u, in0=idx8, scalar1=31, scalar2=None,
                    op0=Alu.bitwise_and,
                )
                loc = sel_pool.tile([P, KS], bf16, tag="loc")
                nc.scalar.copy(loc, loc_u)
                m_ = mid_pool.tile([P, KS, CHF], bf16, tag="m_")
                nc.vector.tensor_tensor(
                    out=m_,
                    in0=jhalf[:, None, :].broadcast_to([P, KS, CHF]),
                    in1=loc[:, :, None].broadcast_to([P, KS, CHF]),
                    op=Alu.is_equal,
                )
                mx = mid_pool.tile([P, KS, CHF], bf16, tag="mx")
                nc.vector.tensor_tensor(out=mx, in0=m_, in1=chunks, op=Alu.mult)
                # topA2[p, (t,k,c)] = sum_j mx[p, (t,k), 2j+c]
                topA2 = sel_pool.tile([P, KS, 2], bf16, tag="topA2")
                st["topA2"] = topA2
                with nc.allow_low_precision("one-hot select, single nonzero"):
                    nc.vector.tensor_reduce(
                        out=topA2,
                        in_=mx.rearrange("p tk (j c) -> p tk c j", c=2),
                        axis=mybir.AxisListType.X,
                        op=Alu.add,
                    )


                return st

            def stage_b2(s, st):
                """Per-tile MLP + weighted sum + store."""
                topA2, rden, wscr = st["topA2"], st["rden"], st["wscr"]
                # ---- per-tile MLP + weighted sum ----
                for j in range(ST):
                    t = s * ST + j
                    taT_ps = tpool.tile([2 * K, P], bf16, tag="tp_ps")
                    nc.tensor.transpose(
                        taT_ps,
                        topA2[:, ds(K * j, K), :].rearrange("p k c -> p (k c)"),
                        identb,
                    )
                    taT = mlp_pool.tile([2 * K, P], bf16, tag="taT")
                    nc.scalar.copy(taT, taT_ps)
                    psum1 = ppool.tile([EMB, K * P], f32, tag="pmm")
                    for k in range(K):
                        nc.tensor.matmul(
                            psum1[:, ts(k, P)], w1T[:, ts(k, EMB)], taT
                        )
                    h1 = mlp_pool.tile([EMB, K * P], bf16, tag="h1")
                    nc.scalar.activation(h1, psum1, Act.Gelu, bias=b1c)

                    psum2 = ppool.tile([EMB, K * P], f32, tag="pmm")
                    nc.tensor.matmul(psum2[:, :512], w2T, h1[:, :512])
                    nc.tensor.matmul(psum2[:, 512:], w2T, h1[:, 512:])
                    topE = mlp_pool.tile([EMB, K * P], bf16, tag="topE")
                    nc.scalar.activation(topE, psum2, Act.Gelu, bias=b2c)

                    wrep = mlp_pool.tile([EMB, K, P], bf16, tag="wrep")
                    nc.sync.dma_start(
                        out=wrep,
                        in_=wscr[None, ds(K * j, K), :].broadcast_to([EMB, K, P]),
                    )
                    wtmp = mlp_pool.tile([EMB, K * P], bf16, tag="wtmp")
                    nc.gpsimd.tensor_tensor(
                        out=wtmp, in0=topE,
                        in1=wrep.rearrange("e k p -> e (k p)"), op=Alu.mult
                    )
                    f1 = mlp_pool.tile([EMB, 4 * P], bf16, tag="f1")
                    nc.gpsimd.tensor_tensor(
                        out=f1, in0=wtmp[:, : 4 * P], in1=wtmp[:, 4 * P :],
                        op=Alu.add,
                    )
                    f2 = mlp_pool.tile([EMB, 2 * P], bf16, tag="f2")
                    nc.gpsimd.tensor_tensor(
                        out=f2, in0=f1[:, : 2 * P], in1=f1[:, 2 * P :], op=Alu.add
                    )
                    outT = mlp_pool.tile([EMB, P], bf16, tag="outT")
                    nc.gpsimd.tensor_tensor(
                        out=outT, in0=f2[:, :P], in1=f2[:, P:], op=Alu.add
                    )

                    outQ_ps = oqpool.tile([P, EMB], bf16, tag="outq_ps")
                    nc.tensor.transpose(outQ_ps, outT, identb)
                    out_sb = mlp_pool.tile([P, EMB], f32, tag="out_sb")
                    nc.scalar.mul(out_sb, outQ_ps, mul=rden[:, j])

                    nc.sync.dma_start(out=out_ext[ts(t, P), :], in_=out_sb)


            state = {}
            for s in range(n_super + 2):
                if s < n_super:
                    state[s] = stage_a(s)
                if s >= 1 and s - 1 < n_super:
                    stage_b1(s - 1, state[s - 1])
                if s >= 2:
                    stage_b2(s - 2, state.pop(s - 2))

    nc.compile()
    return nc


def make_in_map(gl_shard, anc_shard, prep, n_tiles):
    m = {
        "ngl": shard_queries(gl_shard, n_tiles),
        "anci": anc_shard,
        "ancp": np.ascontiguousarray(anc_shard.transpose(2, 0, 1)),
    }
    m.update(prep)
    return m


_GRAPH_CACHE = {}
_TRACE = False       # set by test harnesses to capture a profile
LAST_RESULT = None   # BassKernelResults of the most recent kernel() call


def kernel(Gl_cur, ancL, W1, b1, W2, b2):
    global LAST_RESULT
    from concourse.bass_utils import run_bass_kernel_spmd

    Gl_cur = np.ascontiguousarray(Gl_cur, dtype=np.float32)
    ancL = np.ascontiguousarray(ancL, dtype=np.float32)
    prep = host_prep(
        np.asarray(W1, dtype=np.float32),
        np.asarray(b1, dtype=np.float32),
        np.asarray(W2, dtype=np.float32),
        np.asarray(b2, dtype=np.float32),
    )

    if "nc" not in _GRAPH_CACHE:
        _GRAPH_CACHE["nc"] = build_graph(NS_FULL)
    nc = _GRAPH_CACHE["nc"]

    in_maps = []
    for i in range(NCORES):
        sl = slice(i * BP, (i + 1) * BP)
        in_maps.append(make_in_map(Gl_cur[sl], ancL[sl], prep, NT_FULL))
    res = run_bass_kernel_spmd(nc, in_maps, list(range(NCORES)), trace=_TRACE)
    LAST_RESULT = res
    return np.concatenate([res.results[i]["out"] for i in range(NCORES)], axis=0)


# revision 18
# speedup vs baseline: 1.1748x; 1.1748x over previous
"""Trainium2 Bass kernel for nn_AnchorKNN_OnlyL (retrieval_knn).

Per query b (32768 total): among its own 2048 2-D anchors, find the 8 nearest
(L2), run each through a 2->128->128 erf-GELU MLP, and combine with
softmax(d2_top / tau) weights.

Sharding: pure data parallel over queries -- 4096 queries per NeuronCore,
params replicated, no collectives.

v2 design (supertiles of 4 x 128 queries):
  - host passes anchors twice: planar [2, bp, M] for contiguous distance
    reads, interleaved [bp, M, 2] for the 256B-chunk gather
  - selection path in bf16: ACT squares -> DVE STT add -> MAX8/FIND_INDEX8
  - all K-wide small ops batched 4 tiles wide ([P, 32])
  - wrapped gather index list built with a DRAM bounce (2 small DMAs)
    instead of the PE transpose fold
  - one dma_gather of 4096 indices per supertile
  - softmax weights broadcast to [EMB, K*P] via ones[1,EMB] matmul into PSUM
  - MLP layer1 as 8 contraction-2 bf16 matmuls from a single batched
    topAT transpose; layer2 bf16 128x128; k-fold via strided tensor_reduce
"""

import sys

sys.path.insert(0, "/opt/trn_rl_repo")

import numpy as np

B, M, EMB, K = 32768, 2048, 128, 8
TAU = 0.3
NCORES = 8
BP = B // NCORES          # queries per core
P = 128                   # partitions / queries per tile
NT_FULL = BP // P         # 32 tiles per core
ST = 4                    # tiles per supertile
NS_FULL = NT_FULL // ST   # 8 supertiles per core
CHF = 64                  # floats per gathered chunk (256 B = 32 anchors)
NCHUNK = (M * 2) // CHF   # 64 chunks per query row


def host_prep(W1, b1, W2, b2):
    """Derived replicated parameters computed host-side."""
    w1T = np.ascontiguousarray(W1.T)                      # [2, EMB]
    w1big = np.zeros((2 * K, K * EMB), dtype=np.float32)  # 8 block stationaries
    for k in range(K):
        w1big[2 * k : 2 * k + 2, k * EMB : (k + 1) * EMB] = w1T
    w2t = np.ascontiguousarray(W2.T)                      # [EMB, EMB]
    return {
        "w1t": w1big,
        "w2t": w2t,
        "b1c": np.ascontiguousarray(b1.reshape(EMB, 1)),
        "b2c": np.ascontiguousarray(b2.reshape(EMB, 1)),
        "repident": _repident(),
    }


def _repident():
    r = np.zeros((16, P), dtype=np.float32)   # S[r, p] = (p % 16 == r)
    for p in range(P):
        r[p % 16, p] = 1.0
    return r


def shard_queries(Gl_cur, n_tiles):
    """[bp, 2] -> negated, tile-major layout [P, n_tiles, 2]."""
    g = Gl_cur.reshape(n_tiles, P, 2).transpose(1, 0, 2)
    return np.ascontiguousarray(-g)


def build_graph(n_super=NS_FULL):
    import concourse.bass as bass
    import concourse.mybir as mybir
    from concourse.bass import ds, ts
    from concourse.masks import make_identity
    from concourse.tile import TileContext

    f32 = mybir.dt.float32
    bf16 = mybir.dt.bfloat16
    i16 = mybir.dt.int16
    u16 = mybir.dt.uint16
    Alu = mybir.AluOpType
    Act = mybir.ActivationFunctionType

    from concourse import bacc

    n_tiles = n_super * ST
    bp = n_tiles * P
    KS = K * ST               # 32: batched top-k width per supertile
    nc = bacc.Bacc(None, dynamic_dma_scratch_size=16384, num_swdge_queues=4)

    ngl_ext = nc.declare_dram_parameter("ngl", [P, n_tiles, 2], f32, isOutput=False)
    ancp_ext = nc.declare_dram_parameter("ancp", [2, bp, M], f32, isOutput=False)
    anci_ext = nc.declare_dram_parameter("anci", [bp, M, 2], f32, isOutput=False)
    w1t_ext = nc.declare_dram_parameter("w1t", [2 * K, K * EMB], f32, isOutput=False)
    w2t_ext = nc.declare_dram_parameter("w2t", [EMB, EMB], f32, isOutput=False)
    b1_ext = nc.declare_dram_parameter("b1c", [EMB, 1], f32, isOutput=False)
    b2_ext = nc.declare_dram_parameter("b2c", [EMB, 1], f32, isOutput=False)
    repident_ext = nc.declare_dram_parameter("repident", [16, P], f32, isOutput=False)
    out_ext = nc.declare_dram_parameter("out", [bp, EMB], f32, isOutput=True)

    with TileContext(nc) as tc:
        with (
            tc.tile_pool(name="consts", bufs=1) as consts,
            tc.tile_pool(name="xy", bufs=6) as xy_pool,
            tc.tile_pool(name="sq", bufs=6) as sq_pool,
            tc.tile_pool(name="s16", bufs=4) as s_pool,
            tc.tile_pool(name="sel", bufs=4) as sel_pool,
            tc.tile_pool(name="mid", bufs=2) as mid_pool,
            tc.tile_pool(name="mlp", bufs=5) as mlp_pool,
            tc.tile_pool(name="psum_mm", bufs=2, space="PSUM") as ppool,
            tc.tile_pool(name="psum_tp", bufs=1, space="PSUM") as tpool,
            tc.tile_pool(name="psum_f2", bufs=1, space="PSUM") as f2pool,
            tc.tile_pool(name="psum_fold", bufs=1, space="PSUM") as foldpool,
            tc.tile_pool(name="psum_oq", bufs=1, space="PSUM") as oqpool,
            tc.tile_pool(name="dram", bufs=4, space="DRAM") as dram_pool,
        ):
            # ---------------- constants ----------------
            ident = consts.tile([P, P], f32)
            make_identity(nc, ident)
            identb = consts.tile([P, P], bf16)
            nc.scalar.copy(identb, ident)

            w1sb = consts.tile([2 * K, K * EMB], f32)
            nc.sync.dma_start(out=w1sb, in_=w1t_ext[:, :])
            w1T = consts.tile([2 * K, K * EMB], bf16)
            nc.scalar.copy(w1T, w1sb)

            w2sb = consts.tile([EMB, EMB], f32)
            nc.sync.dma_start(out=w2sb, in_=w2t_ext[:, :])
            w2T = consts.tile([EMB, EMB], bf16)
            nc.scalar.copy(w2T, w2sb)

            b1c = consts.tile([EMB, 1], f32)
            nc.sync.dma_start(out=b1c, in_=b1_ext[:, :])
            b2c = consts.tile([EMB, 1], f32)
            nc.sync.dma_start(out=b2c, in_=b2_ext[:, :])

            repident = consts.tile([16, P], f32)
            nc.sync.dma_start(out=repident, in_=repident_ext[:, :])

            ngl = consts.tile([P, n_tiles, 2], f32)
            nc.sync.dma_start(out=ngl, in_=ngl_ext[:, :, :])

            # jhalf[p, j] = j // 2 for j in [0, 64): pair id of float j in chunk
            jhalf_u = consts.tile([P, CHF], u16)
            nc.gpsimd.iota(jhalf_u, pattern=[[1, CHF // 2], [0, 2]], channel_multiplier=0)
            jhalf = consts.tile([P, CHF], bf16)
            nc.scalar.copy(jhalf, jhalf_u)

            # qiota[p, t*K+k] = 64*p + 8192*t (chunk-row base within supertile)
            qiota = consts.tile([P, KS], u16)
            nc.gpsimd.iota(
                qiota, pattern=[[P * NCHUNK, ST], [0, K]], channel_multiplier=NCHUNK
            )

            def stage_a(s):
                """Per supertile: load 4 tiles, distances, top-8 each."""
                vals8 = sel_pool.tile([P, KS], bf16, tag="vals8")
                idx8 = sel_pool.tile([P, KS], u16, tag="idx8")
                for j in range(ST):
                    t = s * ST + j
                    xt = xy_pool.tile([P, M], f32, tag="xy")
                    nc.sync.dma_start(out=xt, in_=ancp_ext[0, ts(t, P)])
                    yt = xy_pool.tile([P, M], f32, tag="xy")
                    nc.sync.dma_start(out=yt, in_=ancp_ext[1, ts(t, P)])

                    tx2 = sq_pool.tile([P, M], bf16, tag="sq")
                    ty2 = sq_pool.tile([P, M], bf16, tag="sq")
                    nc.scalar.activation(tx2, xt, Act.Square, bias=ngl[:, t, 0:1])
                    nc.scalar.activation(ty2, yt, Act.Square, bias=ngl[:, t, 1:2])

                    st_ = s_pool.tile([P, M], bf16, tag="s16")
                    nc.vector.scalar_tensor_tensor(
                        out=st_, in0=tx2, scalar=-1.0, in1=ty2,
                        op0=Alu.mult, op1=Alu.subtract,
                    )
                    nc.vector.max(out=vals8[:, ts(j, K)], in_=st_)
                    nc.vector.max_index(
                        out=idx8[:, ts(j, K)], in_max=vals8[:, ts(j, K)], in_values=st_
                    )
                return dict(vals8=vals8, idx8=idx8)

            def stage_b1(s, st):
                """Batched: index math, fold, gather, softmax, extraction."""
                vals8, idx8 = st["vals8"], st["idx8"]

                # ---- gather index list ----
                chunk = sel_pool.tile([P, KS], u16, tag="chunk")
                nc.vector.tensor_scalar(
                    out=chunk, in0=idx8, scalar1=5, scalar2=None,
                    op0=Alu.logical_shift_right,
                )
                chunkq = sel_pool.tile([P, KS], u16, tag="chunkq")
                nc.vector.tensor_tensor(out=chunkq, in0=chunk, in1=qiota, op=Alu.add)

                # ---- softmax weights (tanh-exp), batched [P, 32] ----
                v = vals8.rearrange("p (t k) -> p t k", t=ST)
                sub = sel_pool.tile([P, ST, K], f32, tag="sub")
                nc.vector.tensor_tensor(
                    out=sub, in0=v,
                    in1=v[:, :, 7:8].broadcast_to([P, ST, K]),
                    op=Alu.subtract,
                )
                th = sel_pool.tile([P, KS], f32, tag="th")
                nc.scalar.activation(
                    th, sub.rearrange("p t k -> p (t k)"), Act.Tanh,
                    scale=-1.0 / (2.0 * TAU),
                )
                den = sel_pool.tile([P, KS], f32, tag="den")
                nc.vector.tensor_scalar(
                    out=den, in0=th, scalar1=-1.0, scalar2=1.0,
                    op0=Alu.mult, op1=Alu.add,
                )
                rden8 = sel_pool.tile([P, KS], f32, tag="rden8")
                nc.vector.reciprocal(rden8, den)

                # cmb = [chunkq as f32 | exp8]: one transpose serves both the
                # gather-index fold and the softmax-weight broadcast
                cmb = sel_pool.tile([P, 2 * KS], f32, tag="cmb")
                nc.scalar.copy(cmb[:, :KS], chunkq)
                nc.vector.scalar_tensor_tensor(
                    out=cmb[:, ds(KS, KS)], in0=th, scalar=1.0, in1=rden8,
                    op0=Alu.add, op1=Alu.mult,
                )
                denom = sel_pool.tile([P, ST, 1], f32, tag="denom")
                nc.vector.tensor_reduce(
                    out=denom,
                    in_=cmb[:, ds(KS, KS)].rearrange("p (t k) -> p t k", t=ST),
                    axis=mybir.AxisListType.X,
                    op=Alu.add,
                )
                rden = sel_pool.tile([P, ST, 1], f32, tag="rden")
                nc.vector.reciprocal(rden, denom)
                st["rden"] = rden

                # ---- fold: [P, 64] -> [64, P] -> wrapped idx list + expT ----
                fold_a = foldpool.tile([EMB, KS * 8], f32, tag="fold")
                cqeT_ps = fold_a[ds(0, 2 * KS), ds(0, P)]
                nc.tensor.transpose(cqeT_ps, cmb, ident)
                cqT_sb = sel_pool.tile([KS, P], f32, tag="cqT_sb")
                nc.scalar.copy(cqT_sb, cqeT_ps[ds(0, KS), :])
                expT_sb = sel_pool.tile([KS, P], bf16, tag="expT_sb")
                nc.scalar.copy(expT_sb, cqeT_ps[ds(KS, KS), :])
                wscr = dram_pool.tile([KS, P], bf16, tag="wscr")
                nc.sync.dma_start(out=wscr, in_=expT_sb)
                st["wscr"] = wscr

                m16_ps = f2pool.tile([16, KS * 8], f32, tag="f2")
                for jj in range(8):
                    nc.tensor.transpose(
                        m16_ps[:, ds(KS * jj, KS)],
                        cqT_sb[:, ds(16 * jj, 16)],
                        ident[ds(0, KS), ds(0, KS)],
                    )
                m16_sb = sel_pool.tile([16, KS * 8], f32, tag="m16_sb")
                nc.scalar.copy(m16_sb, m16_ps)
                wps = foldpool.tile([EMB, KS * 8], f32, tag="fold")
                nc.tensor.matmul(
                    wps,
                    repident,
                    m16_sb.rearrange("r (j tk) -> r tk j", j=8),
                )
                wrapped = sel_pool.tile([P, KS * 8], i16, tag="wrapped")
                nc.scalar.copy(wrapped, wps)

                # ---- gather 256B chunks, one 1024-idx call per tile ----
                chunks = mid_pool.tile([P, KS, CHF], f32, tag="chunks")
                for j in range(ST):
                    nc.gpsimd.dma_gather(
                        out_ap=chunks[:, ds(K * j, K), :],
                        in_ap=anci_ext[ts(s, ST * P)].rearrange(
                            "p (g r) c -> (p g) (r c)", r=CHF // 2
                        ),
                        idxs_ap=wrapped[:, ds(K * 8 * j, K * 8)],
                        num_idxs=P * K,
                        num_idxs_reg=P * K,
                        elem_size=CHF,
                        queue_num=j,
                    )

                # ---- chunk-local selection mask ----
                loc_u = sel_pool.tile([P, KS], u16, tag="loc_u")
                nc.vector.tensor_scalar(
                    out=loc_u, in0=idx8, scalar1=31, scalar2=None,
                    op0=Alu.bitwise_and,
                )
                loc = sel_pool.tile([P, KS], bf16, tag="loc")
                nc.scalar.copy(loc, loc_u)
                m_ = mid_pool.tile([P, KS, CHF], bf16, tag="m_")
                nc.vector.tensor_tensor(
                    out=m_,
                    in0=jhalf[:, None, :].broadcast_to([P, KS, CHF]),
                    in1=loc[:, :, None].broadcast_to([P, KS, CHF]),
                    op=Alu.is_equal,
                )
                mx = mid_pool.tile([P, KS, CHF], bf16, tag="mx")
                nc.vector.tensor_tensor(out=mx, in0=m_, in1=chunks, op=Alu.mult)
                # topA2[p, (t,k,c)] = sum_j mx[p, (t,k), 2j+c]
                topA2 = sel_pool.tile([P, KS, 2], bf16, tag="topA2")
                st["topA2"] = topA2
                with nc.allow_low_precision("one-hot select, single nonzero"):
                    nc.vector.tensor_reduce(
                        out=topA2,
                        in_=mx.rearrange("p tk (j c) -> p tk c j", c=2),
                        axis=mybir.AxisListType.X,
                        op=Alu.add,
                    )


                return st

            def stage_b2(s, st):
                """Per-tile MLP + weighted sum + store."""
                topA2, rden, wscr = st["topA2"], st["rden"], st["wscr"]
                # ---- per-tile MLP + weighted sum ----
                for j in range(ST):
                    t = s * ST + j
                    taT_ps = tpool.tile([2 * K, P], bf16, tag="tp_ps")
                    nc.tensor.transpose(
                        taT_ps,
                        topA2[:, ds(K * j, K), :].rearrange("p k c -> p (k c)"),
                        identb,
                    )
                    taT = mlp_pool.tile([2 * K, P], bf16, tag="taT")
                    nc.scalar.copy(taT, taT_ps)
                    psum1 = ppool.tile([EMB, K * P], f32, tag="pmm")
                    for k in range(K):
                        nc.tensor.matmul(
                            psum1[:, ts(k, P)], w1T[:, ts(k, EMB)], taT
                        )
                    h1 = mlp_pool.tile([EMB, K * P], bf16, tag="h1")
                    nc.scalar.activation(h1, psum1, Act.Gelu, bias=b1c)

                    psum2 = ppool.tile([EMB, K * P], f32, tag="pmm")
                    nc.tensor.matmul(psum2[:, :512], w2T, h1[:, :512])
                    nc.tensor.matmul(psum2[:, 512:], w2T, h1[:, 512:])
                    topE = mlp_pool.tile([EMB, K * P], bf16, tag="topE")
                    nc.scalar.activation(topE, psum2, Act.Gelu, bias=b2c)

                    wrep = mlp_pool.tile([EMB, K, P], bf16, tag="wrep")
                    nc.sync.dma_start(
                        out=wrep,
                        in_=wscr[None, ds(K * j, K), :].broadcast_to([EMB, K, P]),
                    )
                    wtmp = mlp_pool.tile([EMB, K * P], bf16, tag="wtmp")
                    nc.vector.tensor_tensor(
                        out=wtmp, in0=topE,
                        in1=wrep.rearrange("e k p -> e (k p)"), op=Alu.mult
                    )
                    outT = mlp_pool.tile([EMB, P], bf16, tag="outT")
                    with nc.allow_low_precision("8-way weighted sum in bf16"):
                        nc.vector.tensor_reduce(
                            out=outT[:, :, None],
                            in_=wtmp.rearrange("e (k p) -> e p k", k=K),
                            axis=mybir.AxisListType.X,
                            op=Alu.add,
                        )

                    outQ_ps = oqpool.tile([P, EMB], bf16, tag="outq_ps")
                    nc.tensor.transpose(outQ_ps, outT, identb)
                    out_sb = mlp_pool.tile([P, EMB], f32, tag="out_sb")
                    nc.scalar.mul(out_sb, outQ_ps, mul=rden[:, j])

                    nc.sync.dma_start(out=out_ext[ts(t, P), :], in_=out_sb)


            state = {}
            for s in range(n_super + 2):
                if s >= 2:
                    stage_b2(s - 2, state.pop(s - 2))
                if s < n_super:
                    state[s] = stage_a(s)
                if s >= 1 and s - 1 < n_super:
                    stage_b1(s - 1, state[s - 1])

    nc.compile()
    return nc


def make_in_map(gl_shard, anc_shard, prep, n_tiles):
    m = {
        "ngl": shard_queries(gl_shard, n_tiles),
        "anci": anc_shard,
        "ancp": np.ascontiguousarray(anc_shard.transpose(2, 0, 1)),
    }
    m.update(prep)
    return m


_GRAPH_CACHE = {}
_TRACE = False       # set by test harnesses to capture a profile
LAST_RESULT = None   # BassKernelResults of the most recent kernel() call


def kernel(Gl_cur, ancL, W1, b1, W2, b2):
    global LAST_RESULT
    from concourse.bass_utils import run_bass_kernel_spmd

    Gl_cur = np.ascontiguousarray(Gl_cur, dtype=np.float32)
    ancL = np.ascontiguousarray(ancL, dtype=np.float32)
    prep = host_prep(
        np.asarray(W1, dtype=np.float32),
        np.asarray(b1, dtype=np.float32),
        np.asarray(W2, dtype=np.float32),
        np.asarray(b2, dtype=np.float32),
    )

    if "nc" not in _GRAPH_CACHE:
        _GRAPH_CACHE["nc"] = build_graph(NS_FULL)
    nc = _GRAPH_CACHE["nc"]

    in_maps = []
    for i in range(NCORES):
        sl = slice(i * BP, (i + 1) * BP)
        in_maps.append(make_in_map(Gl_cur[sl], ancL[sl], prep, NT_FULL))
    res = run_bass_kernel_spmd(nc, in_maps, list(range(NCORES)), trace=_TRACE)
    LAST_RESULT = res
    return np.concatenate([res.results[i]["out"] for i in range(NCORES)], axis=0)
